# revision 16
# baseline (speedup 1.0000x reference)
"""3D Haar wavelet transform (2x2x2, causal temporal pad) on 8 Trainium2 cores.

Input  x: (2, 3, 33, 512, 512) fp32
Output y: (2, 24, 17, 256, 256) fp32   (channel = 3*s + c, s = subband)

Sharding: pure data parallel over H — core ci handles input rows
[64*ci, 64*ci+64) i.e. output rows [32*ci, 32*ci+32).

Key idea: the host pre-splits ALL THREE Haar pair axes (temporal i,
h-parity j, w-parity k) into the SBUF partition dim:
    p = i*64 + k*32 + j*16 + q        (q = row-pair index, 16 per 32-row
                                       group, 2 groups rg per core)
so the whole 2x2x2 Haar transform collapses into ONE 128x128 matmul
per tile (weights (+-0.3536)*delta(q,q'), contraction over (i,k,j)):
    out[m=(s,q), f] = sum_p W[p, m] * in[p, f],   s = 4*di+2*dj+dk.
No DVE pre-stage at all; DVE/ACT only evacuate PSUM -> fp16 SBUF.

Wire format is fp16 both directions (rel err ~4e-4, gate is 2e-2),
halving HBM traffic vs fp32: 13.4MB in + 13.4MB out per core
-> ~75us DMA roofline at ~360 GB/s.

Per-core pipeline, per (b, c) macro-step (6 total):
  2 contiguous ~1.1MB DMAs in -> A[128, 8704] fp16   (free = (rg,T',w'))
  17 matmuls [128,128]x[128,512] fp16 -> PSUM fp32 (8 banks)
  17 PSUM evacuations alternating DVE / ACT -> C[128, 8704] fp16
  2 contiguous ~1.1MB DMAs out (separate HWDGE queue via nc.scalar)
Host reorders y' -> y (subband-major channels) and casts to fp32.

Measured (2026-08-09): 259785ns baseline -> ~78-85us this design
(run-to-run noise ~+-4us; best observed 78540ns).
Variants that regressed: quarter-size (0.5MB) DMAs = 94.9us; separate
64-partition T'=0 head DMAs (2.9% less traffic) = 85.5us (DMA_15
straggles); extra edge-split DMAs for fill/drain = neutral-to-worse.
"""

import numpy as np

import concourse.bacc as bacc
import concourse.mybir as mybir
from concourse import tile
from concourse.bass_utils import run_bass_kernel_spmd

P = 128
B_, C_, T_, H_, W_ = 2, 3, 33, 512, 512
NCORES = 8
HC = H_ // NCORES          # 64 input rows per core
RG = 2                     # row groups of 32 per core
Q = 16                     # row pairs per group
TP = (T_ + 1) // 2         # 17 output frames
WP = W_ // 2               # 256 output cols
FREE = RG * TP * WP        # 8704 free elements per partition per (b,c)
SCALE = float(np.float32(0.3536))
F16 = mybir.dt.float16
F32 = mybir.dt.float32
MM_N = 512                 # matmul free-dim chunk (one PSUM bank)


def _haar_matrix() -> np.ndarray:
    """W[p, m]: p = i*64 + k*32 + j*16 + q, m = (4di+2dj+dk)*16 + q,
    val SCALE * (-1)^(i*di + j*dj + k*dk)."""
    W = np.zeros((P, P), dtype=np.float32)
    for i in range(2):
        for k in range(2):
            for j in range(2):
                for q in range(Q):
                    p = i * 64 + k * 32 + j * 16 + q
                    for di in range(2):
                        for dj in range(2):
                            for dk in range(2):
                                m = (4 * di + 2 * dj + dk) * Q + q
                                W[p, m] = SCALE * (-1.0) ** (i * di + j * dj + k * dk)
    return W.astype(np.float16)


def build_nc():
    nc = bacc.Bacc("TRN2", target_bir_lowering=False, debug=False)
    # x': [b, c, p, (rg, T', w')] host-pretransposed fp16, pad baked in
    x_d = nc.dram_tensor("x", [B_, C_, P, FREE], F16, kind="ExternalInput")
    # y': [b, c, m, (rg, T', w')] fp16, m = s*16 + q
    y_d = nc.dram_tensor("y", [B_, C_, P, FREE], F16, kind="ExternalOutput")
    w_d = nc.inline_tensor(_haar_matrix(), name="haar_w")

    chunks = [(off, min(MM_N, FREE - off)) for off in range(0, FREE, MM_N)]
    # chunk-aligned split points for half-tile DMA granularity
    # (FREE=8704 is 17 chunks of 512; halves [0:4096) and [4096:8704))
    CUTS = [0, 4096, FREE]

    with tile.TileContext(nc) as tc:
        with (
            tc.tile_pool(name="wpool", bufs=1) as wpool,
            tc.tile_pool(name="apool", bufs=3) as apool,
            tc.tile_pool(name="cpool", bufs=3) as cpool,
            tc.tile_pool(name="psum", bufs=8, space="PSUM") as psum_pool,
        ):
            w_sb = wpool.tile([P, P], F16)
            nc.sync.dma_start(out=w_sb[:], in_=w_d[:])

            for b in range(B_):
                for c in range(C_):
                    a = apool.tile([P, FREE], F16, tag="a")
                    for lo, hi in zip(CUTS[:-1], CUTS[1:]):
                        nc.sync.dma_start(
                            out=a[:, lo:hi], in_=x_d[b, c, :, lo:hi]
                        )
                    cout = cpool.tile([P, FREE], F16, tag="c")
                    sub = 0
                    for off, n in chunks:
                        ps = psum_pool.tile([P, MM_N], F32)
                        nc.tensor.matmul(
                            ps[:, 0:n], w_sb[:], a[:, off : off + n],
                            start=True, stop=True,
                        )
                        # alternate PSUM evacuation between DVE and ACT
                        if sub % 2 == 0:
                            nc.vector.tensor_copy(
                                out=cout[:, off : off + n], in_=ps[:, 0:n]
                            )
                        else:
                            nc.scalar.copy(
                                out=cout[:, off : off + n], in_=ps[:, 0:n]
                            )
                        sub += 1
                        if off + n in CUTS:
                            lo = CUTS[CUTS.index(off + n) - 1]
                            nc.scalar.dma_start(
                                out=y_d[b, c, :, lo : off + n],
                                in_=cout[:, lo : off + n],
                            )
    nc.compile()
    return nc


_NC_CACHE = None


def _get_nc():
    global _NC_CACHE
    if _NC_CACHE is None:
        _NC_CACHE = build_nc()
    return _NC_CACHE


# xp[tp] = x[max(tp-1, 0)] (causal pad); pair (T', i) reads xp[2T'+i]
_TIDX = np.maximum(np.arange(2 * TP) - 1, 0)


def make_in_maps(x: np.ndarray) -> list[dict]:
    xh = np.ascontiguousarray(x, dtype=np.float32).astype(np.float16)
    xp = xh[:, :, _TIDX, :, :]                       # [2,3,34,512,512]
    in_maps = []
    for ci in range(NCORES):
        xc = xp[:, :, :, HC * ci : HC * (ci + 1), :]  # view [2,3,34,64,512]
        # split axes: T=(T',i), h=(rg,q,j), w=(w',k)
        xc = xc.reshape(B_, C_, TP, 2, RG, Q, 2, WP, 2)
        # -> [b, c, i, k, j, q, rg, T', w']
        xc = xc.transpose(0, 1, 3, 8, 6, 5, 4, 2, 7)
        xc = np.ascontiguousarray(xc).reshape(B_, C_, P, FREE)
        in_maps.append({"x": xc})
    return in_maps


def assemble_output(results) -> np.ndarray:
    y8 = np.empty((B_, 8, C_, TP, H_ // 2, WP), dtype=np.float32)
    for ci in range(NCORES):
        yc = results[ci]["y"]                         # [2,3,128,8704] fp16
        yc = yc.reshape(B_, C_, 8, Q, RG, TP, WP)     # [b,c,s,q,rg,T',w']
        yc = yc.transpose(0, 2, 1, 5, 4, 3, 6)        # [b,s,c,T',rg,q,w']
        y8[:, :, :, :, 32 * ci : 32 * (ci + 1), :] = yc.reshape(
            B_, 8, C_, TP, 2 * Q, WP
        )
    return y8.reshape(B_, 8 * C_, TP, H_ // 2, WP)


def kernel(x: np.ndarray) -> np.ndarray:
    assert x.shape == (B_, C_, T_, H_, W_), x.shape
    nc = _get_nc()
    in_maps = make_in_maps(x)
    res = run_bass_kernel_spmd(nc, in_maps, core_ids=list(range(NCORES)))
    return assemble_output(res.results)
